# revision 4
# baseline (speedup 1.0000x reference)
"""Additive (Bahdanau-style) attention scores kernel for Trainium2.

Computes softmax(We @ tanh(query@Wq.T + keys@Wk.T), axis=-1) for
B=32, S=2048, D=1024, data-parallel over batch across 8 NeuronCores.

Per-core layout strategy ("layout B"):
  - keys tiles [s,d] are PE-transposed (exact, fp32) to [d,s]
  - pkT[e, s] = WkT[d,e].T @ keysT[d,s], f32r matmuls (full PE rate)
  - energy = tanh(pkT + pq[e]) on ACT with per-partition bias
  - scores[s] = sum_e We[e] * energy[e, s] as rank-1 accumulating matmuls
  - softmax over s on DVE/ACT
"""

import numpy as np
from contextlib import ExitStack

import concourse.bass as bass
import concourse.mybir as mybir
import concourse.tile as tile
from concourse import bacc
from concourse.bass_utils import run_bass_kernel_spmd
from concourse.masks import make_identity

f32 = mybir.dt.float32
f32r = mybir.dt.float32r

B, S, D, E = 32, 2048, 1024, 1024
NCORES = 8
BL = B // NCORES      # 4 batches per core
S_BLK = 512
N_SBLK = S // S_BLK   # 4
DT = D // 128         # 8 d-tiles
ET = E // 128         # 8 e-tiles
ST = S_BLK // 128     # 4 s-subtiles per block

_CACHE: dict = {}


def _build_nc():
    nc = bacc.Bacc("TRN2", target_bir_lowering=False, debug=False, num_devices=NCORES)

    keys_d = nc.dram_tensor("keys", [BL, S, D], f32, kind="ExternalInput")
    qT_d = nc.dram_tensor("queryT", [D, BL], f32, kind="ExternalInput")
    wkT_d = nc.dram_tensor("wkT", [D, E], f32, kind="ExternalInput")
    wqT_d = nc.dram_tensor("wqT", [D, E], f32, kind="ExternalInput")
    weT_d = nc.dram_tensor("weT", [E, 1], f32, kind="ExternalInput")
    out_d = nc.dram_tensor("out", [BL, S], f32, kind="ExternalOutput")

    with tile.TileContext(nc) as tc, ExitStack() as ctx:
        wpool = ctx.enter_context(tc.tile_pool(name="weights", bufs=1))
        raw_pool = ctx.enter_context(tc.tile_pool(name="raw", bufs=2))
        kT_pool = ctx.enter_context(tc.tile_pool(name="kT", bufs=2))
        en_pool = ctx.enter_context(tc.tile_pool(name="en", bufs=4))
        small = ctx.enter_context(tc.tile_pool(name="small", bufs=1))

        ps_tr = ctx.enter_context(tc.tile_pool(name="ps_tr", bufs=2, space="PSUM"))
        ps_pk = ctx.enter_context(tc.tile_pool(name="ps_pk", bufs=2, space="PSUM"))
        ps_sc = ctx.enter_context(tc.tile_pool(name="ps_sc", bufs=2, space="PSUM"))
        ps_pq = ctx.enter_context(tc.tile_pool(name="ps_pq", bufs=1, space="PSUM"))

        # ---- weights (cast to f32r during SWDGE DMA) ----
        wkT_sb = wpool.tile([128, DT, E], f32r)
        wqT_sb = wpool.tile([128, DT, E], f32r)
        wkT_re = wkT_d[:].rearrange("(dt p) e -> p dt e", p=128)
        wqT_re = wqT_d[:].rearrange("(dt p) e -> p dt e", p=128)
        for dt in range(DT):
            nc.gpsimd.dma_start(wkT_sb[:, dt], wkT_re[:, dt])
            nc.gpsimd.dma_start(wqT_sb[:, dt], wqT_re[:, dt])
        weT_sb = wpool.tile([128, ET], f32r)
        for et in range(ET):
            nc.gpsimd.dma_start(weT_sb[:, et : et + 1],
                                weT_d[et * 128 : (et + 1) * 128, :])
        qT_sb = wpool.tile([128, DT, BL], f32r)
        qT_re = qT_d[:].rearrange("(dt p) b -> p dt b", p=128)
        for dt in range(DT):
            nc.gpsimd.dma_start(qT_sb[:, dt], qT_re[:, dt])

        ident = wpool.tile([128, 128], f32)
        make_identity(nc, ident)

        # ---- pq[e, b] = sum_d WqT[d,e] * queryT[d,b] ----
        pq_sb = small.tile([128, ET, BL], f32)
        for et in range(ET):
            pq_ps = ps_pq.tile([128, BL], f32)
            for dt in range(DT):
                nc.tensor.matmul(pq_ps,
                                 lhsT=wqT_sb[:, dt, et * 128 : (et + 1) * 128],
                                 rhs=qT_sb[:, dt],
                                 start=(dt == 0), stop=(dt == DT - 1))
            nc.vector.tensor_copy(pq_sb[:, et], pq_ps)

        # engine writes must start at partition 0 (or 32-aligned), so scores
        # accumulate on partition 0 and are DMA-scattered to [BL, S] later
        scores_tmp = small.tile([1, BL * S], f32)

        # ---- main loop over (batch, s-block) ----
        for b in range(BL):
            for sblk in range(N_SBLK):
                s0 = sblk * S_BLK
                raw = raw_pool.tile([128, ST, D], f32)
                for st in range(ST):
                    nc.sync.dma_start(
                        raw[:, st],
                        keys_d[b, s0 + st * 128 : s0 + (st + 1) * 128, :])

                # transpose to keysT[d, s]
                kT = kT_pool.tile([128, DT, S_BLK], f32r)
                for dt in range(DT):
                    tp = ps_tr.tile([128, S_BLK], f32)
                    for st in range(ST):
                        nc.tensor.transpose(
                            tp[:, st * 128 : (st + 1) * 128],
                            raw[:, st, dt * 128 : (dt + 1) * 128],
                            ident)
                    nc.vector.tensor_copy(kT[:, dt], tp)

                # pk -> tanh -> We contraction
                sc_ps = ps_sc.tile([1, S_BLK], f32)
                for et in range(ET):
                    pk_ps = ps_pk.tile([128, S_BLK], f32)
                    for dt in range(DT):
                        nc.tensor.matmul(
                            pk_ps,
                            lhsT=wkT_sb[:, dt, et * 128 : (et + 1) * 128],
                            rhs=kT[:, dt],
                            start=(dt == 0), stop=(dt == DT - 1))
                    en = en_pool.tile([128, S_BLK], f32r)
                    nc.scalar.activation(en, pk_ps,
                                         mybir.ActivationFunctionType.Tanh,
                                         bias=pq_sb[:, et, b : b + 1],
                                         scale=1.0)
                    nc.tensor.matmul(sc_ps,
                                     lhsT=weT_sb[:, et : et + 1],
                                     rhs=en,
                                     start=(et == 0), stop=(et == ET - 1),
                                     skip_group_check=True)
                off = b * S + s0
                nc.vector.tensor_copy(scores_tmp[0 : 1, off : off + S_BLK], sc_ps)

        # ---- softmax over s, all BL batches at once ----
        scores_sb = small.tile([BL, S], f32)
        for b in range(BL):
            nc.sync.dma_start(scores_sb[b : b + 1, :],
                              scores_tmp[0 : 1, b * S : (b + 1) * S])
        mx = small.tile([BL, 1], f32)
        nc.vector.reduce_max(mx, scores_sb, axis=mybir.AxisListType.X)
        neg_mx = small.tile([BL, 1], f32)
        nc.vector.tensor_scalar_mul(neg_mx, mx, -1.0)
        ex = small.tile([BL, S], f32)
        sumx = small.tile([BL, 1], f32)
        nc.scalar.activation(ex, scores_sb,
                             mybir.ActivationFunctionType.Exp,
                             bias=neg_mx, scale=1.0, accum_out=sumx)
        rinv = small.tile([BL, 1], f32)
        nc.vector.reciprocal(rinv, sumx)
        outb = small.tile([BL, S], f32)
        nc.vector.tensor_scalar_mul(outb, ex, rinv)
        nc.sync.dma_start(out_d[:], outb)

    nc.compile()
    return nc


def _get_nc():
    if "nc" not in _CACHE:
        _CACHE["nc"] = _build_nc()
    return _CACHE["nc"]


def kernel(query, keys, Wq, Wk, We, _return_raw=False, _trace=False):
    query = np.asarray(query, dtype=np.float32)
    keys = np.asarray(keys, dtype=np.float32)
    Wq = np.asarray(Wq, dtype=np.float32)
    Wk = np.asarray(Wk, dtype=np.float32)
    We = np.asarray(We, dtype=np.float32)

    wkT = np.ascontiguousarray(Wk.T)
    wqT = np.ascontiguousarray(Wq.T)
    weT = np.ascontiguousarray(We.reshape(1, E).T)

    in_maps = []
    for c in range(NCORES):
        bs = slice(c * BL, (c + 1) * BL)
        in_maps.append({
            "keys": np.ascontiguousarray(keys[bs]),
            "queryT": np.ascontiguousarray(query[bs].T),
            "wkT": wkT,
            "wqT": wqT,
            "weT": weT,
        })

    nc = _get_nc()
    res = run_bass_kernel_spmd(nc, in_maps, list(range(NCORES)), trace=_trace)
    out = np.concatenate([res.results[c]["out"] for c in range(NCORES)], axis=0)
    if _return_raw:
        return out, res
    return out


# revision 5
# speedup vs baseline: 1.0760x; 1.0760x over previous
"""Additive (Bahdanau-style) attention scores kernel for Trainium2.

Computes softmax(We @ tanh(query@Wq.T + keys@Wk.T), axis=-1) for
B=32, S=2048, D=1024, data-parallel over batch across 8 NeuronCores.

Per-core strategy (v2, fp16 datapath):
  - keys are cast f32->fp16 during the SWDGE load, then transposed to
    [d, s] by the XBAR DMA-transpose engine (no PE transposes at all)
  - pkT[e, s] = WkT[d,e].T @ keysT[d,s] in fp16 (full PE rate, fp32 acc)
  - energy = tanh(pkT + pq[e]) on ACT with per-partition bias
  - scores[s] = sum_e We[e] * energy[e, s] as rank-1 accumulating matmuls
  - softmax over s on DVE/ACT
"""

import numpy as np
from contextlib import ExitStack

import concourse.bass as bass
import concourse.mybir as mybir
import concourse.tile as tile
from concourse import bacc
from concourse.bass_utils import run_bass_kernel_spmd

f32 = mybir.dt.float32
fp16 = mybir.dt.float16

B, S, D, E = 32, 2048, 1024, 1024
NCORES = 8
BL = B // NCORES      # 4 batches per core
S_BLK = 512
N_SBLK = S // S_BLK   # 4
DT = D // 128         # 8 d-tiles
ET = E // 128         # 8 e-tiles
ST = S_BLK // 128     # 4 s-subtiles per block

_CACHE: dict = {}


def _build_nc():
    nc = bacc.Bacc("TRN2", target_bir_lowering=False, debug=False, num_devices=NCORES)

    keys_d = nc.dram_tensor("keys", [BL, S, D], f32, kind="ExternalInput")
    qT_d = nc.dram_tensor("queryT", [D, BL], f32, kind="ExternalInput")
    wkT_d = nc.dram_tensor("wkT", [D, E], f32, kind="ExternalInput")
    wqT_d = nc.dram_tensor("wqT", [D, E], f32, kind="ExternalInput")
    weT_d = nc.dram_tensor("weT", [E, 1], f32, kind="ExternalInput")
    out_d = nc.dram_tensor("out", [BL, S], f32, kind="ExternalOutput")

    with tile.TileContext(nc) as tc, ExitStack() as ctx:
        wpool = ctx.enter_context(tc.tile_pool(name="weights", bufs=1))
        raw_pool = ctx.enter_context(tc.tile_pool(name="raw", bufs=2))
        kT_pool = ctx.enter_context(tc.tile_pool(name="kT", bufs=2))
        en_pool = ctx.enter_context(tc.tile_pool(name="en", bufs=4))
        small = ctx.enter_context(tc.tile_pool(name="small", bufs=1))

        ps_pk = ctx.enter_context(tc.tile_pool(name="ps_pk", bufs=2, space="PSUM"))
        ps_sc = ctx.enter_context(tc.tile_pool(name="ps_sc", bufs=2, space="PSUM"))
        ps_pq = ctx.enter_context(tc.tile_pool(name="ps_pq", bufs=1, space="PSUM"))

        # ---- weights (cast to fp16 during SWDGE DMA) ----
        wkT_sb = wpool.tile([128, DT, E], fp16)
        wkT_re = wkT_d[:].rearrange("(dt p) e -> p dt e", p=128)
        for dt in range(DT):
            nc.gpsimd.dma_start(wkT_sb[:, dt], wkT_re[:, dt])
        qT_sb = wpool.tile([128, DT, BL], fp16)
        qT_re = qT_d[:].rearrange("(dt p) b -> p dt b", p=128)
        for dt in range(DT):
            nc.gpsimd.dma_start(qT_sb[:, dt], qT_re[:, dt])
        wqT_sb = wpool.tile([128, DT, E], fp16)
        wqT_re = wqT_d[:].rearrange("(dt p) e -> p dt e", p=128)
        for dt in range(DT):
            nc.gpsimd.dma_start(wqT_sb[:, dt], wqT_re[:, dt])
        weT_sb = wpool.tile([128, ET], fp16)
        for et in range(ET):
            nc.gpsimd.dma_start(weT_sb[:, et : et + 1],
                                weT_d[et * 128 : (et + 1) * 128, :])

        # ---- pq: layout-A matmul [b, e] then XBAR transpose to [e-part, b] ----
        # pq_row[b, e] = sum_d qT[d, b] * wqT[d, e]; lhsT = qT tile (4 cols)
        pq_row = wpool.tile([16, E], fp16)   # rows 0..3 used; 16 for XBAR align
        nc.gpsimd.memset(pq_row, 0.0)
        for half in range(2):
            pq_ps = ps_pq.tile([BL, 512], f32)
            for dt in range(DT):
                nc.tensor.matmul(pq_ps,
                                 lhsT=qT_sb[:, dt],
                                 rhs=wqT_sb[:, dt, half * 512 : (half + 1) * 512],
                                 start=(dt == 0), stop=(dt == DT - 1))
            nc.vector.tensor_copy(pq_row[:BL, half * 512 : (half + 1) * 512], pq_ps)
        pq_sb = wpool.tile([128, ET, 16], fp16)
        nc.sync.dma_start_transpose(pq_sb, pq_row)

        # engine writes must start at partition 0, so scores accumulate on
        # partition 0 and are DMA-scattered to [BL, S] afterwards
        scores_tmp = small.tile([1, BL * S], f32)

        # ---- main loop over (batch, s-block) ----
        for b in range(BL):
            for sblk in range(N_SBLK):
                s0 = sblk * S_BLK
                raw16 = raw_pool.tile([128, ST, D], fp16)
                for st in range(ST):
                    nc.gpsimd.dma_start(
                        raw16[:, st],
                        keys_d[b, s0 + st * 128 : s0 + (st + 1) * 128, :])

                # XBAR transpose [128(s), 1024(d)] -> [128(d), 8(dt), 128(s)]
                kT = kT_pool.tile([128, DT, S_BLK], fp16)
                for st in range(ST):
                    nc.sync.dma_start_transpose(
                        kT[:, :, st * 128 : (st + 1) * 128], raw16[:, st])

                # pk -> tanh -> We contraction
                sc_ps = ps_sc.tile([1, S_BLK], f32)
                for et in range(ET):
                    pk_ps = ps_pk.tile([128, S_BLK], f32)
                    for dt in range(DT):
                        nc.tensor.matmul(
                            pk_ps,
                            lhsT=wkT_sb[:, dt, et * 128 : (et + 1) * 128],
                            rhs=kT[:, dt],
                            start=(dt == 0), stop=(dt == DT - 1))
                    en = en_pool.tile([128, S_BLK], fp16)
                    nc.scalar.activation(en, pk_ps,
                                         mybir.ActivationFunctionType.Tanh,
                                         bias=pq_sb[:, et, b : b + 1],
                                         scale=1.0)
                    nc.tensor.matmul(sc_ps,
                                     lhsT=weT_sb[:, et : et + 1],
                                     rhs=en,
                                     start=(et == 0), stop=(et == ET - 1),
                                     skip_group_check=True)
                off = b * S + s0
                nc.vector.tensor_copy(scores_tmp[0 : 1, off : off + S_BLK], sc_ps)

        # ---- softmax over s, all BL batches at once ----
        scores_sb = small.tile([BL, S], f32)
        for b in range(BL):
            nc.sync.dma_start(scores_sb[b : b + 1, :],
                              scores_tmp[0 : 1, b * S : (b + 1) * S])
        mx = small.tile([BL, 1], f32)
        nc.vector.reduce_max(mx, scores_sb, axis=mybir.AxisListType.X)
        neg_mx = small.tile([BL, 1], f32)
        nc.vector.tensor_scalar_mul(neg_mx, mx, -1.0)
        ex = small.tile([BL, S], f32)
        sumx = small.tile([BL, 1], f32)
        nc.scalar.activation(ex, scores_sb,
                             mybir.ActivationFunctionType.Exp,
                             bias=neg_mx, scale=1.0, accum_out=sumx)
        rinv = small.tile([BL, 1], f32)
        nc.vector.reciprocal(rinv, sumx)
        outb = small.tile([BL, S], f32)
        nc.vector.tensor_scalar_mul(outb, ex, rinv)
        nc.sync.dma_start(out_d[:], outb)

    nc.compile()
    return nc


def _get_nc():
    if "nc" not in _CACHE:
        _CACHE["nc"] = _build_nc()
    return _CACHE["nc"]


def kernel(query, keys, Wq, Wk, We, _return_raw=False, _trace=False):
    query = np.asarray(query, dtype=np.float32)
    keys = np.asarray(keys, dtype=np.float32)
    Wq = np.asarray(Wq, dtype=np.float32)
    Wk = np.asarray(Wk, dtype=np.float32)
    We = np.asarray(We, dtype=np.float32)

    wkT = np.ascontiguousarray(Wk.T)
    wqT = np.ascontiguousarray(Wq.T)
    weT = np.ascontiguousarray(We.reshape(1, E).T)

    in_maps = []
    for c in range(NCORES):
        bs = slice(c * BL, (c + 1) * BL)
        in_maps.append({
            "keys": np.ascontiguousarray(keys[bs]),
            "queryT": np.ascontiguousarray(query[bs].T),
            "wkT": wkT,
            "wqT": wqT,
            "weT": weT,
        })

    nc = _get_nc()
    res = run_bass_kernel_spmd(nc, in_maps, list(range(NCORES)), trace=_trace)
    out = np.concatenate([res.results[c]["out"] for c in range(NCORES)], axis=0)
    if _return_raw:
        return out, res
    return out


# revision 6
# speedup vs baseline: 1.2518x; 1.1634x over previous
"""Additive (Bahdanau-style) attention scores kernel for Trainium2.

Computes softmax(We @ tanh(query@Wq.T + keys@Wk.T), axis=-1) for
B=32, S=2048, D=1024, data-parallel over batch across 8 NeuronCores.

Per-core strategy (v3, fp16 datapath):
  - weights arrive pre-transposed and fp16 from the host (replicated,
    tiny); keys are cast f32->fp16 during the SWDGE load, then
    transposed to [d, s] by the XBAR DMA-transpose engine
  - pkT[e, s] = WkT[d,e].T @ keysT[d,s] in fp16 (full PE rate, fp32 acc)
  - energy = tanh(pkT + pq[e]) on ACT with per-partition bias
  - scores[s] = sum_e We[e] * energy[e, s] as rank-1 accumulating
    matmuls, deferred into the NEXT block's pk stream so the PE never
    waits on ACT
  - per-batch softmax over s (DVE/ACT), overlapped with the main loop
"""

import numpy as np
from contextlib import ExitStack

import concourse.bass as bass
import concourse.mybir as mybir
import concourse.tile as tile
from concourse import bacc
from concourse.bass_utils import run_bass_kernel_spmd

f32 = mybir.dt.float32
fp16 = mybir.dt.float16

B, S, D, E = 32, 2048, 1024, 1024
NCORES = 8
BL = B // NCORES      # 4 batches per core
S_BLK = 512
N_SBLK = S // S_BLK   # 4
DT = D // 128         # 8 d-tiles
ET = E // 128         # 8 e-tiles
ST = S_BLK // 128     # 4 s-subtiles per block

_CACHE: dict = {}


def _build_nc():
    nc = bacc.Bacc("TRN2", target_bir_lowering=False, debug=False, num_devices=NCORES)

    keys_d = nc.dram_tensor("keys", [BL, S, D], f32, kind="ExternalInput")
    qT_d = nc.dram_tensor("queryT16", [D, BL], fp16, kind="ExternalInput")
    wkT_d = nc.dram_tensor("wkT16", [D, E], fp16, kind="ExternalInput")
    wqT_d = nc.dram_tensor("wqT16", [D, E], fp16, kind="ExternalInput")
    weT_d = nc.dram_tensor("weT16", [E, 1], fp16, kind="ExternalInput")
    out_d = nc.dram_tensor("out", [BL, S], f32, kind="ExternalOutput")

    with tile.TileContext(nc) as tc, ExitStack() as ctx:
        wpool = ctx.enter_context(tc.tile_pool(name="weights", bufs=1))
        raw_pool = ctx.enter_context(tc.tile_pool(name="raw", bufs=2))
        kT_pool = ctx.enter_context(tc.tile_pool(name="kT", bufs=2))
        en_pool = ctx.enter_context(tc.tile_pool(name="en", bufs=12))
        small = ctx.enter_context(tc.tile_pool(name="small", bufs=1))
        sm_pool = ctx.enter_context(tc.tile_pool(name="smx", bufs=2))

        ps_pk = ctx.enter_context(tc.tile_pool(name="ps_pk", bufs=3, space="PSUM"))
        ps_sc = ctx.enter_context(tc.tile_pool(name="ps_sc", bufs=2, space="PSUM"))
        ps_pq = ctx.enter_context(tc.tile_pool(name="ps_pq", bufs=1, space="PSUM"))

        # ---- weights: plain fp16 HWDGE loads (parallel with keys casts) ----
        wkT_sb = wpool.tile([128, DT, E], fp16)
        nc.sync.dma_start(wkT_sb, wkT_d[:].rearrange("(dt p) e -> p dt e", p=128))
        qT_sb = wpool.tile([128, DT, BL], fp16)
        nc.sync.dma_start(qT_sb, qT_d[:].rearrange("(dt p) b -> p dt b", p=128))
        wqT_sb = wpool.tile([128, DT, E], fp16)
        nc.sync.dma_start(wqT_sb, wqT_d[:].rearrange("(dt p) e -> p dt e", p=128))
        weT_sb = wpool.tile([128, ET], fp16)
        nc.sync.dma_start(weT_sb,
                          weT_d[:].rearrange("(et p) one -> p (et one)", p=128))

        # ---- pq: layout-A matmul [b, e], then XBAR transpose to [e, b] ----
        pq_row = wpool.tile([16, E], fp16)   # rows 0..3 used; 16 for XBAR align
        nc.gpsimd.memset(pq_row, 0.0)
        for half in range(2):
            pq_ps = ps_pq.tile([BL, 512], f32)
            for dt in range(DT):
                nc.tensor.matmul(pq_ps,
                                 lhsT=qT_sb[:, dt],
                                 rhs=wqT_sb[:, dt, half * 512 : (half + 1) * 512],
                                 start=(dt == 0), stop=(dt == DT - 1))
            nc.vector.tensor_copy(pq_row[:BL, half * 512 : (half + 1) * 512], pq_ps)
        pq_sb = wpool.tile([128, ET, 16], fp16)
        nc.scalar.dma_start_transpose(pq_sb, pq_row)

        # engine writes must start at partition 0, so scores accumulate on
        # partition 0 (one row per core) and per-batch softmax reads slices
        scores_tmp = small.tile([1, BL * S], f32)

        def emit_softmax(b):
            """Softmax of batch b over scores_tmp[0, b*S:(b+1)*S] -> out."""
            row = scores_tmp[0:1, b * S : (b + 1) * S]
            mx = sm_pool.tile([1, 1], f32, tag="mx")
            nc.vector.reduce_max(mx, row, axis=mybir.AxisListType.X)
            neg_mx = sm_pool.tile([1, 1], f32, tag="negmx")
            nc.vector.tensor_scalar_mul(neg_mx, mx, -1.0)
            ex = sm_pool.tile([1, S], f32, tag="ex")
            sumx = sm_pool.tile([1, 1], f32, tag="sumx")
            nc.scalar.activation(ex, row, mybir.ActivationFunctionType.Exp,
                                 bias=neg_mx, scale=1.0, accum_out=sumx)
            rinv = sm_pool.tile([1, 1], f32, tag="rinv")
            nc.vector.reciprocal(rinv, sumx)
            outr = sm_pool.tile([1, S], f32, tag="outr")
            nc.vector.tensor_scalar_mul(outr, ex, rinv)
            nc.sync.dma_start(out_d[b : b + 1, :], outr)

        # ---- main loop over (batch, s-block) ----
        pending = None  # deferred We-contraction of the previous block

        blocks = [(b, sblk) for b in range(BL) for sblk in range(N_SBLK)]
        for b, sblk in blocks:
            s0 = sblk * S_BLK
            raw16 = raw_pool.tile([128, ST, D], fp16)
            for st in range(ST):
                nc.gpsimd.dma_start(
                    raw16[:, st],
                    keys_d[b, s0 + st * 128 : s0 + (st + 1) * 128, :])

            # XBAR transpose [128(s), 1024(d)] -> [128(d), 8(dt), 128(s)]
            kT = kT_pool.tile([128, DT, S_BLK], fp16)
            for st in range(ST):
                nc.sync.dma_start_transpose(
                    kT[:, :, st * 128 : (st + 1) * 128], raw16[:, st])

            en_tiles = []
            for et in range(ET):
                pk_ps = ps_pk.tile([128, S_BLK], f32)
                for dt in range(DT):
                    nc.tensor.matmul(
                        pk_ps,
                        lhsT=wkT_sb[:, dt, et * 128 : (et + 1) * 128],
                        rhs=kT[:, dt],
                        start=(dt == 0), stop=(dt == DT - 1))
                en = en_pool.tile([128, S_BLK], fp16)
                nc.scalar.activation(en, pk_ps,
                                     mybir.ActivationFunctionType.Tanh,
                                     bias=pq_sb[:, et, b : b + 1],
                                     scale=1.0)
                en_tiles.append(en)
                if et == 1 and pending is not None:
                    pending()
                    pending = None

            def make_pending(b_, sblk_, tiles):
                def emit():
                    sc_ps = ps_sc.tile([1, S_BLK], f32)
                    for et_ in range(ET):
                        nc.tensor.matmul(sc_ps,
                                         lhsT=weT_sb[:, et_ : et_ + 1],
                                         rhs=tiles[et_],
                                         start=(et_ == 0), stop=(et_ == ET - 1),
                                         skip_group_check=True)
                    off = b_ * S + sblk_ * S_BLK
                    nc.vector.tensor_copy(
                        scores_tmp[0 : 1, off : off + S_BLK], sc_ps)
                    if sblk_ == N_SBLK - 1:
                        emit_softmax(b_)
                return emit

            pending = make_pending(b, sblk, en_tiles)

        pending()

    nc.compile()
    return nc


def _get_nc():
    if "nc" not in _CACHE:
        _CACHE["nc"] = _build_nc()
    return _CACHE["nc"]


def kernel(query, keys, Wq, Wk, We, _return_raw=False, _trace=False):
    query = np.asarray(query, dtype=np.float32)
    keys = np.asarray(keys, dtype=np.float32)
    Wq = np.asarray(Wq, dtype=np.float32)
    Wk = np.asarray(Wk, dtype=np.float32)
    We = np.asarray(We, dtype=np.float32)

    wkT = np.ascontiguousarray(Wk.T).astype(np.float16)
    wqT = np.ascontiguousarray(Wq.T).astype(np.float16)
    weT = np.ascontiguousarray(We.reshape(1, E).T).astype(np.float16)

    in_maps = []
    for c in range(NCORES):
        bs = slice(c * BL, (c + 1) * BL)
        in_maps.append({
            "keys": np.ascontiguousarray(keys[bs]),
            "queryT16": np.ascontiguousarray(query[bs].T).astype(np.float16),
            "wkT16": wkT,
            "wqT16": wqT,
            "weT16": weT,
        })

    nc = _get_nc()
    res = run_bass_kernel_spmd(nc, in_maps, list(range(NCORES)), trace=_trace)
    out = np.concatenate([res.results[c]["out"] for c in range(NCORES)], axis=0)
    if _return_raw:
        return out, res
    return out
